# Initial kernel scaffold
#
"""Trainium2 Bass kernel for nn_CustomLSTM: 1000-step LSTM, batch 128,
input 128, hidden 1024, 50 categories, on 8 NeuronCores (one trn2 chip).

Sharding: model-parallel over the hidden dimension. Core p owns hidden block
p (128 of 1024 units) for all four gates, with the full batch of 128 on the
PE partition axis, so every recurrent matmul runs with a full 128x128
stationary (8x better PE utilization than batch-parallel, which would leave
M=16). The recurrence is strictly sequential; each step ends with an
AllGather of the per-core h-slices, PE-transposed to [hidden, batch] and
cast to bf16 (halves exchange bytes; cell state c and all elementwise math
stay fp32, PSUM accumulation stays fp32). The x-projection and the bias add
(as a rank-1 ones @ bias matmul) are fused into the same PSUM accumulation
and are issued inside the AllGather wait window. The final
h_S @ W_out.T partial product is computed on-device per core; the host sums
the 8 partials and adds b_out.

kernel(**inputs) takes the FULL unsharded inputs keyed as in setup_inputs()
and returns the FULL [128, 50] float32 output.
"""

from contextlib import ExitStack

import numpy as np
import ml_dtypes

from concourse import bass, mybir
from concourse.bass_utils import run_bass_kernel_spmd

N_CORES = 8
B = 128      # batch
H = 1024     # hidden
HP = H // N_CORES
NG = 4 * HP  # gate columns per core, order f|i|o|g
I = 128      # input features
S = 1000     # sequence length
F32 = mybir.dt.float32
BF16 = mybir.dt.bfloat16
SIG = mybir.ActivationFunctionType.Sigmoid
TANH = mybir.ActivationFunctionType.Tanh


def _build_lstm(xt_depth: int = 8):
    nc = bass.Bass(num_devices=N_CORES, target_bir_lowering=False, debug=False)

    xT = nc.declare_dram_parameter("xT", [S, I, B], F32, isOutput=False)
    wh = nc.declare_dram_parameter("wh", [H, NG], BF16, isOutput=False)
    wx = nc.declare_dram_parameter("wx", [I, NG], F32, isOutput=False)
    brow = nc.declare_dram_parameter("brow", [1, NG], F32, isOutput=False)
    wout = nc.declare_dram_parameter("wout", [HP, 50], F32, isOutput=False)
    ident = nc.declare_dram_parameter("ident", [128, 128], F32, isOutput=False)
    ones = nc.declare_dram_parameter("ones", [1, B], F32, isOutput=False)
    y = nc.declare_dram_parameter("y", [B, 50], F32, isOutput=True)

    in_bounce = nc.dram_tensor("in_bounce", [HP, B], BF16)
    out_bounce = nc.dram_tensor("out_bounce", [H, B], BF16, addr_space="Shared")

    with ExitStack() as _es:
        wh_sb = _es.enter_context(nc.sbuf_tensor("wh_sb", [128, 8 * NG], BF16))
        wx_sb = _es.enter_context(nc.sbuf_tensor("wx_sb", [128, NG], F32))
        b_sb = _es.enter_context(nc.sbuf_tensor("b_sb", [1, NG], F32))
        wout_sb = _es.enter_context(nc.sbuf_tensor("wout_sb", [128, 50], F32))
        id_sb = _es.enter_context(nc.sbuf_tensor("id_sb", [128, 128], F32))
        ones_sb = _es.enter_context(nc.sbuf_tensor("ones_sb", [1, B], F32))
        xt_sb = _es.enter_context(
            nc.sbuf_tensor("xt_sb", [128, xt_depth * B], F32)
        )
        c_sb = _es.enter_context(nc.sbuf_tensor("c_sb", [128, HP], F32))
        hT_all = _es.enter_context(nc.sbuf_tensor("hT_all", [128, H], BF16))
        hT_send = _es.enter_context(nc.sbuf_tensor("hT_send", [128, B], BF16))
        hT_fin = _es.enter_context(nc.sbuf_tensor("hT_fin", [128, B], F32))
        fio_sb = _es.enter_context(nc.sbuf_tensor("fio_sb", [128, 3 * HP], F32))
        f_sb = fio_sb[:, 0:HP]
        i_sb = fio_sb[:, HP : 2 * HP]
        o_sb = fio_sb[:, 2 * HP : 3 * HP]
        g_sb = _es.enter_context(nc.sbuf_tensor("g_sb", [128, HP], F32))
        fc_sb = _es.enter_context(nc.sbuf_tensor("fc_sb", [128, HP], F32))
        ig_sb = _es.enter_context(nc.sbuf_tensor("ig_sb", [128, HP], F32))
        h_sb = _es.enter_context(nc.sbuf_tensor("h_sb", [128, HP], F32))
        tc_sb = _es.enter_context(nc.sbuf_tensor("tc_sb", [128, HP], F32))
        y_sb = _es.enter_context(nc.sbuf_tensor("y_sb", [128, 50], F32))
        g_ps0 = _es.enter_context(nc.psum_tensor("g_ps0", [128, NG], F32))
        g_ps1 = _es.enter_context(nc.psum_tensor("g_ps1", [128, NG], F32))
        hT_ps = _es.enter_context(nc.psum_tensor("hT_ps", [128, B], F32))
        y_ps = _es.enter_context(nc.psum_tensor("y_ps", [128, 50], F32))
        dma_w_sem = _es.enter_context(nc.semaphore("dma_w_sem"))
        dma_x_sem = _es.enter_context(nc.semaphore("dma_x_sem"))
        dma_out_sem = _es.enter_context(nc.semaphore("dma_out_sem"))
        cc_sem = _es.enter_context(nc.semaphore("cc_sem"))
        dma_in_sem = _es.enter_context(nc.semaphore("dma_in_sem"))
        pe_g_sem = _es.enter_context(nc.semaphore("pe_g_sem"))
        pe_x_sem = _es.enter_context(nc.semaphore("pe_x_sem"))
        pe_tr_sem = _es.enter_context(nc.semaphore("pe_tr_sem"))
        act_sem = _es.enter_context(nc.semaphore("act_sem"))
        dve_c_sem = _es.enter_context(nc.semaphore("dve_c_sem"))
        dve_h_sem = _es.enter_context(nc.semaphore("dve_h_sem"))
        dve_hT_sem = _es.enter_context(nc.semaphore("dve_hT_sem"))
        dve_y_sem = _es.enter_context(nc.semaphore("dve_y_sem"))
        init_sem = _es.enter_context(nc.semaphore("init_sem"))
        block = _es.enter_context(nc.Block())
        g_ps = [g_ps0, g_ps1]

        def xt_tile(t):
            s = (t - 1) % xt_depth
            return xt_sb[:, s * B : (s + 1) * B]

        def wh_tile(k):
            return wh_sb[:, k * NG : (k + 1) * NG]

        def hT_tile(k):
            return hT_all[:, k * B : (k + 1) * B]

        N_INIT = 13

        # ---------------- sync engine: all HWDGE DMA ----------------
        @block.sync
        def _(sync):
            for k in range(8):
                sync.dma_start(
                    out=wh_tile(k), in_=wh[k * 128 : (k + 1) * 128, :]
                ).then_inc(dma_w_sem, 16)
            sync.dma_start(out=wx_sb[:, :], in_=wx[:, :]).then_inc(dma_w_sem, 16)
            sync.dma_start(out=b_sb[:, :], in_=brow[:, :]).then_inc(dma_w_sem, 16)
            sync.dma_start(out=wout_sb[0:HP, :], in_=wout[:, :]).then_inc(
                dma_w_sem, 16
            )
            sync.dma_start(out=id_sb[:, :], in_=ident[:, :]).then_inc(
                dma_w_sem, 16
            )
            sync.dma_start(out=ones_sb[:, :], in_=ones[:, :]).then_inc(
                dma_w_sem, 16
            )
            for t in range(1, min(xt_depth, S) + 1):
                if t >= 2:
                    # keep at most one xT DMA in flight so the threshold
                    # wait on dma_x_sem identifies WHICH load completed
                    sync.wait_ge(dma_x_sem, 16 * (t - 1))
                sync.dma_start(out=xt_tile(t), in_=xT[t - 1]).then_inc(
                    dma_x_sem, 16
                )
            for t in range(1, S + 1):
                j = t + xt_depth
                if j <= S:
                    sync.wait_ge(pe_x_sem, j - xt_depth)
                    sync.wait_ge(dma_x_sem, 16 * (j - 1))
                    sync.dma_start(out=xt_tile(j), in_=xT[j - 1]).then_inc(
                        dma_x_sem, 16
                    )
                if t <= S - 1:
                    sync.wait_ge(dve_hT_sem, t)
                    sync.dma_start(
                        out=in_bounce[:, :], in_=hT_send[:, :]
                    ).then_inc(dma_out_sem, 16)
                    sync.wait_ge(cc_sem, t)
                    # single DMA + single sem inc: two separate DMAs can
                    # complete out of order, which would let the PE's
                    # half-wait pass on the wrong half (stale stationaries)
                    sync.dma_start(
                        out=hT_all.ap().rearrange("p (r b) -> p r b", r=8),
                        in_=out_bounce.ap().rearrange("(r p) b -> p r b", p=128),
                    ).then_inc(dma_in_sem, 16)
            sync.wait_ge(dve_y_sem, 1)
            sync.dma_start(out=y[:, :], in_=y_sb[0:B, :]).then_inc(
                dma_out_sem, 16
            )
            sync.wait_ge(dma_out_sem, 16 * S)

        # ---------------- gpsimd: init + per-step AllGather ----------------
        @block.gpsimd
        def _(g):
            g.memset(c_sb[:, :], 0.0).then_inc(init_sem, 1)
            for t in range(1, S):
                g.wait_ge(dma_out_sem, 16 * t)
                g.collective_compute(
                    "AllGather",
                    mybir.AluOpType.bypass,
                    replica_groups=[list(range(N_CORES))],
                    ins=[in_bounce[:, :].opt()],
                    outs=[out_bounce[:, :].opt()],
                ).then_inc(cc_sem, 1)

        # ---------------- PE ----------------
        @block.tensor
        def _(tensor):
            tensor.wait_ge(dma_w_sem, 16 * N_INIT)

            def xbias(t, stop=False):
                bank = g_ps[t % 2]
                tensor.wait_ge(dma_x_sem, 16 * t)
                tensor.matmul(
                    bank[:, :], xt_tile(t), wx_sb[:, :], start=True, stop=False
                ).then_inc(pe_x_sem, 1)
                return tensor.matmul(
                    bank[:, :], ones_sb[0:1, :], b_sb[0:1, :],
                    start=False, stop=stop,
                )

            xbias(1, stop=True).then_inc(pe_g_sem, 1)
            if S >= 2:
                xbias(2)
            for t in range(2, S + 1):
                # inside the AllGather(t-1) wait window
                tensor.wait_ge(dve_h_sem, t - 1)
                tensor.transpose(hT_ps[:, :], h_sb[:, :], id_sb[:, :]).then_inc(
                    pe_tr_sem, 1
                )
                if t + 1 <= S:
                    xbias(t + 1)
                # recurrent matmuls on the gathered h(t-1)
                tensor.wait_ge(dma_in_sem, 16 * (t - 1))
                for k in range(8):
                    mm = tensor.matmul(
                        g_ps[t % 2][:, :], hT_tile(k), wh_tile(k),
                        start=False, stop=(k == 7),
                    )
                    if k == 7:
                        mm.then_inc(pe_g_sem, 1)
            tensor.wait_ge(dve_h_sem, S)
            tensor.transpose(hT_ps[:, :], h_sb[:, :], id_sb[:, :]).then_inc(
                pe_tr_sem, 1
            )
            tensor.wait_ge(dve_hT_sem, S + 1)
            tensor.matmul(
                y_ps[:, :], hT_fin[:, :], wout_sb[:, :], start=True, stop=True
            ).then_inc(pe_g_sem, 1)

        # ---------------- ACT (scalar): sigmoids + tanhs ----------------
        @block.scalar
        def _(act):
            for t in range(1, S + 1):
                bank = g_ps[t % 2]
                act.wait_ge(pe_g_sem, t)
                act.activation(f_sb, bank[:, 0:128], SIG).then_inc(act_sem, 1)
                act.activation(g_sb[:, :], bank[:, 384:512], TANH).then_inc(
                    act_sem, 1
                )
                act.activation(
                    fio_sb[:, HP : 3 * HP], bank[:, 128:384], SIG
                ).then_inc(act_sem, 1)
                act.wait_ge(dve_c_sem, t)
                act.activation(tc_sb[:, :], c_sb[:, :], TANH).then_inc(
                    act_sem, 1
                )

        # ---------------- DVE (vector): cell update + copies ----------------
        @block.vector
        def _(v):
            v.wait_ge(init_sem, 1)
            for t in range(1, S + 1):
                base = 4 * (t - 1)
                v.wait_ge(act_sem, base + 1)
                v.tensor_mul(fc_sb[:, :], f_sb[:, :], c_sb[:, :])
                v.wait_ge(act_sem, base + 3)
                v.tensor_mul(ig_sb[:, :], i_sb[:, :], g_sb[:, :])
                v.tensor_add(c_sb[:, :], fc_sb[:, :], ig_sb[:, :]).then_inc(
                    dve_c_sem, 1
                )
                v.wait_ge(act_sem, base + 4)
                v.tensor_mul(h_sb[:, :], o_sb[:, :], tc_sb[:, :]).then_inc(
                    dve_h_sem, 1
                )
                v.wait_ge(pe_tr_sem, t)
                v.tensor_copy(hT_send[:, :], hT_ps[:, :]).then_inc(
                    dve_hT_sem, 1
                )
            # fp32 copy of hT_S for the fp32 output projection
            v.tensor_copy(hT_fin[:, :], hT_ps[:, :]).then_inc(dve_hT_sem, 1)
            v.wait_ge(pe_g_sem, S + 1)
            v.tensor_copy(y_sb[:, :], y_ps[:, :]).then_inc(dve_y_sem, 1)

    return nc


def _prep_inputs(x, W_ii, W_hi, b_ii, W_if, W_hf, b_if, W_ig, W_hg, b_ig,
                 W_io, W_ho, b_io, W_out, b_out):
    """Per-core inputs. Gate column order f|i|o|g (sigmoid block contiguous);
    core p gets hidden slice [p*128, (p+1)*128) of every gate."""
    x = np.ascontiguousarray(np.asarray(x, np.float32))
    xT = np.ascontiguousarray(np.transpose(x, (1, 2, 0)))  # [S, I, B]

    Wx_gates = [W_if, W_ii, W_io, W_ig]
    Wh_gates = [W_hf, W_hi, W_ho, W_hg]
    b_gates = [b_if, b_ii, b_io, b_ig]

    ident = np.eye(128, dtype=np.float32)
    ones_row = np.ones((1, B), np.float32)

    in_maps = []
    for p in range(N_CORES):
        sl = slice(p * HP, (p + 1) * HP)
        wx = np.concatenate(
            [np.asarray(w, np.float32)[:, sl] for w in Wx_gates], axis=1
        )
        whm = np.concatenate(
            [np.asarray(w, np.float32)[:, sl] for w in Wh_gates], axis=1
        ).astype(ml_dtypes.bfloat16)
        brow = np.concatenate(
            [np.asarray(b, np.float32)[sl] for b in b_gates]
        )[None, :]
        woutT = np.ascontiguousarray(np.asarray(W_out, np.float32)[:, sl].T)
        in_maps.append(
            dict(
                xT=xT,
                wh=np.ascontiguousarray(whm),
                wx=np.ascontiguousarray(wx),
                brow=np.ascontiguousarray(brow),
                wout=woutT,
                ident=ident,
                ones=ones_row,
            )
        )
    return in_maps


_CACHED = {}


def _get_nc():
    if "nc" not in _CACHED:
        _CACHED["nc"] = _build_lstm()
    return _CACHED["nc"]


def kernel(**inputs) -> np.ndarray:
    inputs = {k: np.asarray(v) for k, v in inputs.items()}
    in_maps = _prep_inputs(**inputs)
    nc = _get_nc()
    res = run_bass_kernel_spmd(nc, in_maps, core_ids=list(range(N_CORES)))
    y = sum(np.asarray(r["y"], np.float64) for r in res.results)
    y = y + np.asarray(inputs["b_out"], np.float64)
    return y.astype(np.float32)



# revision 1
# speedup vs baseline: 1.0962x; 1.0962x over previous
"""Trainium2 Bass kernel for nn_CustomLSTM: 1000-step LSTM, batch 128,
input 128, hidden 1024, 50 categories, on 8 NeuronCores (one trn2 chip).

Sharding: model-parallel over the hidden dimension. Core p owns hidden block
p (128 of 1024 units) for all four gates, with the full batch of 128 on the
PE partition axis, so every recurrent matmul runs with a full 128x128
stationary (8x better PE utilization than batch-parallel, which would leave
M=16). The recurrence is strictly sequential; each step ends with an
AllGather of the per-core h-slices, PE-transposed to [hidden, batch] and
cast to bf16 (halves exchange bytes; cell state c and all elementwise math
stay fp32, PSUM accumulation stays fp32). The x-projection and the bias add
(as a rank-1 ones @ bias matmul) are fused into the same PSUM accumulation
and are issued inside the AllGather wait window. The final
h_S @ W_out.T partial product is computed on-device per core; the host sums
the 8 partials and adds b_out.

kernel(**inputs) takes the FULL unsharded inputs keyed as in setup_inputs()
and returns the FULL [128, 50] float32 output.
"""

from contextlib import ExitStack

import numpy as np
import ml_dtypes

from concourse import bass, mybir
from concourse.bass_utils import run_bass_kernel_spmd

N_CORES = 8
B = 128      # batch
H = 1024     # hidden
HP = H // N_CORES
NG = 4 * HP  # gate columns per core, order f|i|o|g
I = 128      # input features
S = 1000     # sequence length
F32 = mybir.dt.float32
BF16 = mybir.dt.bfloat16
SIG = mybir.ActivationFunctionType.Sigmoid
TANH = mybir.ActivationFunctionType.Tanh


def _build_lstm(xt_depth: int = 8):
    nc = bass.Bass(num_devices=N_CORES, target_bir_lowering=False, debug=False)

    xT = nc.declare_dram_parameter("xT", [S, I, B], F32, isOutput=False)
    wh = nc.declare_dram_parameter("wh", [H, NG], BF16, isOutput=False)
    wx = nc.declare_dram_parameter("wx", [I, NG], F32, isOutput=False)
    brow = nc.declare_dram_parameter("brow", [1, NG], F32, isOutput=False)
    wout = nc.declare_dram_parameter("wout", [HP, 50], F32, isOutput=False)
    ident = nc.declare_dram_parameter("ident", [128, 128], F32, isOutput=False)
    ones = nc.declare_dram_parameter("ones", [1, B], F32, isOutput=False)
    y = nc.declare_dram_parameter("y", [B, 50], F32, isOutput=True)

    in_bounce = nc.dram_tensor("in_bounce", [HP, B], BF16)
    out_bounce = nc.dram_tensor("out_bounce", [H, B], BF16, addr_space="Shared")

    with ExitStack() as _es:
        wh_sb = _es.enter_context(nc.sbuf_tensor("wh_sb", [128, 8 * NG], BF16))
        wx_sb = _es.enter_context(nc.sbuf_tensor("wx_sb", [128, NG], F32))
        b_sb = _es.enter_context(nc.sbuf_tensor("b_sb", [1, NG], F32))
        wout_sb = _es.enter_context(nc.sbuf_tensor("wout_sb", [128, 50], F32))
        id_sb = _es.enter_context(nc.sbuf_tensor("id_sb", [128, 128], F32))
        ones_sb = _es.enter_context(nc.sbuf_tensor("ones_sb", [1, B], F32))
        xt_sb = _es.enter_context(
            nc.sbuf_tensor("xt_sb", [128, xt_depth * B], F32)
        )
        c_sb = _es.enter_context(nc.sbuf_tensor("c_sb", [128, HP], F32))
        hT_all = _es.enter_context(nc.sbuf_tensor("hT_all", [128, H], BF16))
        hT_send = _es.enter_context(nc.sbuf_tensor("hT_send", [128, B], BF16))
        hT_fin = _es.enter_context(nc.sbuf_tensor("hT_fin", [128, B], F32))
        fio_sb = _es.enter_context(nc.sbuf_tensor("fio_sb", [128, 3 * HP], F32))
        f_sb = fio_sb[:, 0:HP]
        i_sb = fio_sb[:, HP : 2 * HP]
        o_sb = fio_sb[:, 2 * HP : 3 * HP]
        g_sb = _es.enter_context(nc.sbuf_tensor("g_sb", [128, HP], F32))
        fc_sb = _es.enter_context(nc.sbuf_tensor("fc_sb", [128, HP], F32))
        ig_sb = _es.enter_context(nc.sbuf_tensor("ig_sb", [128, HP], F32))
        h_sb = _es.enter_context(nc.sbuf_tensor("h_sb", [128, HP], F32))
        tc_sb = _es.enter_context(nc.sbuf_tensor("tc_sb", [128, HP], F32))
        y_sb = _es.enter_context(nc.sbuf_tensor("y_sb", [128, 50], F32))
        g_ps0 = _es.enter_context(nc.psum_tensor("g_ps0", [128, NG], F32))
        g_ps1 = _es.enter_context(nc.psum_tensor("g_ps1", [128, NG], F32))
        hT_ps = _es.enter_context(nc.psum_tensor("hT_ps", [128, B], F32))
        y_ps = _es.enter_context(nc.psum_tensor("y_ps", [128, 50], F32))
        dma_w_sem = _es.enter_context(nc.semaphore("dma_w_sem"))
        dma_x_sem = _es.enter_context(nc.semaphore("dma_x_sem"))
        dma_out_sem = _es.enter_context(nc.semaphore("dma_out_sem"))
        cc_sem = _es.enter_context(nc.semaphore("cc_sem"))
        dma_in_sem = _es.enter_context(nc.semaphore("dma_in_sem"))
        pe_g_sem = _es.enter_context(nc.semaphore("pe_g_sem"))
        pe_x_sem = _es.enter_context(nc.semaphore("pe_x_sem"))
        pe_tr_sem = _es.enter_context(nc.semaphore("pe_tr_sem"))
        act_sem = _es.enter_context(nc.semaphore("act_sem"))
        dve_c_sem = _es.enter_context(nc.semaphore("dve_c_sem"))
        dve_h_sem = _es.enter_context(nc.semaphore("dve_h_sem"))
        dve_hT_sem = _es.enter_context(nc.semaphore("dve_hT_sem"))
        dve_y_sem = _es.enter_context(nc.semaphore("dve_y_sem"))
        init_sem = _es.enter_context(nc.semaphore("init_sem"))
        block = _es.enter_context(nc.Block())
        g_ps = [g_ps0, g_ps1]

        def xt_tile(t):
            s = (t - 1) % xt_depth
            return xt_sb[:, s * B : (s + 1) * B]

        def wh_tile(k):
            return wh_sb[:, k * NG : (k + 1) * NG]

        def hT_tile(k):
            return hT_all[:, k * B : (k + 1) * B]

        N_INIT = 13

        # ---------------- sync engine: all HWDGE DMA ----------------
        @block.sync
        def _(sync):
            for k in range(8):
                sync.dma_start(
                    out=wh_tile(k), in_=wh[k * 128 : (k + 1) * 128, :]
                ).then_inc(dma_w_sem, 16)
            sync.dma_start(out=wx_sb[:, :], in_=wx[:, :]).then_inc(dma_w_sem, 16)
            sync.dma_start(out=b_sb[:, :], in_=brow[:, :]).then_inc(dma_w_sem, 16)
            sync.dma_start(out=wout_sb[0:HP, :], in_=wout[:, :]).then_inc(
                dma_w_sem, 16
            )
            sync.dma_start(out=id_sb[:, :], in_=ident[:, :]).then_inc(
                dma_w_sem, 16
            )
            sync.dma_start(out=ones_sb[:, :], in_=ones[:, :]).then_inc(
                dma_w_sem, 16
            )
            for t in range(1, min(xt_depth, S) + 1):
                if t >= 2:
                    # keep at most one xT DMA in flight so the threshold
                    # wait on dma_x_sem identifies WHICH load completed
                    sync.wait_ge(dma_x_sem, 16 * (t - 1))
                sync.dma_start(out=xt_tile(t), in_=xT[t - 1]).then_inc(
                    dma_x_sem, 16
                )
            for t in range(1, S + 1):
                j = t + xt_depth
                if j <= S:
                    sync.wait_ge(pe_x_sem, j - xt_depth)
                    sync.wait_ge(dma_x_sem, 16 * (j - 1))
                    sync.dma_start(out=xt_tile(j), in_=xT[j - 1]).then_inc(
                        dma_x_sem, 16
                    )
                if t <= S - 1:
                    sync.wait_ge(dve_hT_sem, t)
                    sync.dma_start(
                        out=in_bounce[:, :], in_=hT_send[:, :]
                    ).then_inc(dma_out_sem, 16)
                    sync.wait_ge(cc_sem, t)
                    # single DMA + single sem inc: two separate DMAs can
                    # complete out of order, which would let the PE's
                    # half-wait pass on the wrong half (stale stationaries)
                    sync.dma_start(
                        out=hT_all.ap().rearrange("p (r b) -> p r b", r=8),
                        in_=out_bounce.ap().rearrange("(r p) b -> p r b", p=128),
                    ).then_inc(dma_in_sem, 16)
            sync.wait_ge(dve_y_sem, 1)
            sync.dma_start(out=y[:, :], in_=y_sb[0:B, :]).then_inc(
                dma_out_sem, 16
            )
            sync.wait_ge(dma_out_sem, 16 * S)

        # ---------------- gpsimd: init + per-step AllGather ----------------
        @block.gpsimd
        def _(g):
            g.memset(c_sb[:, :], 0.0).then_inc(init_sem, 1)
            for t in range(1, S):
                g.wait_ge(dma_out_sem, 16 * t)
                g.collective_compute(
                    "AllGather",
                    mybir.AluOpType.bypass,
                    replica_groups=[list(range(N_CORES))],
                    ins=[in_bounce[:, :].opt()],
                    outs=[out_bounce[:, :].opt()],
                ).then_inc(cc_sem, 1)

        # ---------------- PE ----------------
        @block.tensor
        def _(tensor):
            tensor.wait_ge(dma_w_sem, 16 * N_INIT)

            def xbias(t, stop=False):
                bank = g_ps[t % 2]
                tensor.wait_ge(dma_x_sem, 16 * t)
                tensor.matmul(
                    bank[:, :], xt_tile(t), wx_sb[:, :], start=True, stop=False
                ).then_inc(pe_x_sem, 1)
                return tensor.matmul(
                    bank[:, :], ones_sb[0:1, :], b_sb[0:1, :],
                    start=False, stop=stop,
                )

            xbias(1, stop=True).then_inc(pe_g_sem, 1)
            if S >= 2:
                xbias(2)
            for t in range(2, S + 1):
                # inside the AllGather(t-1) wait window
                tensor.wait_ge(dve_h_sem, t - 1)
                tensor.transpose(hT_ps[:, :], h_sb[:, :], id_sb[:, :]).then_inc(
                    pe_tr_sem, 1
                )
                if t + 1 <= S:
                    xbias(t + 1)
                # recurrent matmuls on the gathered h(t-1)
                tensor.wait_ge(dma_in_sem, 16 * (t - 1))
                for k in range(8):
                    mm = tensor.matmul(
                        g_ps[t % 2][:, :], hT_tile(k), wh_tile(k),
                        start=False, stop=(k == 7),
                    )
                    if k == 7:
                        mm.then_inc(pe_g_sem, 1)
            tensor.wait_ge(dve_h_sem, S)
            tensor.transpose(hT_ps[:, :], h_sb[:, :], id_sb[:, :]).then_inc(
                pe_tr_sem, 1
            )
            tensor.wait_ge(dve_hT_sem, S + 1)
            tensor.matmul(
                y_ps[:, :], hT_fin[:, :], wout_sb[:, :], start=True, stop=True
            ).then_inc(pe_g_sem, 1)

        # ---------------- ACT (scalar): sigmoids + tanhs ----------------
        @block.scalar
        def _(act):
            for t in range(1, S + 1):
                bank = g_ps[t % 2]
                act.wait_ge(pe_g_sem, t)
                act.activation(f_sb, bank[:, 0:128], SIG).then_inc(act_sem, 1)
                act.activation(g_sb[:, :], bank[:, 384:512], TANH).then_inc(
                    act_sem, 1
                )
                act.activation(
                    fio_sb[:, HP : 3 * HP], bank[:, 128:384], SIG
                ).then_inc(act_sem, 1)
                act.wait_ge(dve_c_sem, t)
                act.activation(tc_sb[:, :], c_sb[:, :], TANH).then_inc(
                    act_sem, 1
                )

        # ---------------- DVE (vector): cell update + copies ----------------
        @block.vector
        def _(v):
            v.wait_ge(init_sem, 1)
            for t in range(1, S + 1):
                base = 4 * (t - 1)
                v.wait_ge(act_sem, base + 1)
                v.tensor_mul(fc_sb[:, :], f_sb[:, :], c_sb[:, :])
                v.wait_ge(act_sem, base + 3)
                v.tensor_mul(ig_sb[:, :], i_sb[:, :], g_sb[:, :])
                v.tensor_add(c_sb[:, :], fc_sb[:, :], ig_sb[:, :]).then_inc(
                    dve_c_sem, 1
                )
                v.wait_ge(act_sem, base + 4)
                v.tensor_mul(h_sb[:, :], o_sb[:, :], tc_sb[:, :]).then_inc(
                    dve_h_sem, 1
                )
                v.wait_ge(pe_tr_sem, t)
                v.tensor_copy(hT_send[:, :], hT_ps[:, :]).then_inc(
                    dve_hT_sem, 1
                )
            # fp32 copy of hT_S for the fp32 output projection
            v.tensor_copy(hT_fin[:, :], hT_ps[:, :]).then_inc(dve_hT_sem, 1)
            v.wait_ge(pe_g_sem, S + 1)
            v.tensor_copy(y_sb[:, :], y_ps[:, :]).then_inc(dve_y_sem, 1)

    return nc


def _prep_inputs(x, W_ii, W_hi, b_ii, W_if, W_hf, b_if, W_ig, W_hg, b_ig,
                 W_io, W_ho, b_io, W_out, b_out):
    """Per-core inputs. Gate column order f|i|o|g (sigmoid block contiguous);
    core p gets hidden slice [p*128, (p+1)*128) of every gate."""
    x = np.ascontiguousarray(np.asarray(x, np.float32))
    xT = np.ascontiguousarray(np.transpose(x, (1, 2, 0)))  # [S, I, B]

    Wx_gates = [W_if, W_ii, W_io, W_ig]
    Wh_gates = [W_hf, W_hi, W_ho, W_hg]
    b_gates = [b_if, b_ii, b_io, b_ig]

    ident = np.eye(128, dtype=np.float32)
    ones_row = np.ones((1, B), np.float32)

    in_maps = []
    for p in range(N_CORES):
        sl = slice(p * HP, (p + 1) * HP)
        wx = np.concatenate(
            [np.asarray(w, np.float32)[:, sl] for w in Wx_gates], axis=1
        )
        whm = np.concatenate(
            [np.asarray(w, np.float32)[:, sl] for w in Wh_gates], axis=1
        ).astype(ml_dtypes.bfloat16)
        brow = np.concatenate(
            [np.asarray(b, np.float32)[sl] for b in b_gates]
        )[None, :]
        woutT = np.ascontiguousarray(np.asarray(W_out, np.float32)[:, sl].T)
        in_maps.append(
            dict(
                xT=xT,
                wh=np.ascontiguousarray(whm),
                wx=np.ascontiguousarray(wx),
                brow=np.ascontiguousarray(brow),
                wout=woutT,
                ident=ident,
                ones=ones_row,
            )
        )
    return in_maps


_CACHED = {}


def _get_nc():
    if "nc" not in _CACHED:
        _CACHED["nc"] = _build_lstm()
    return _CACHED["nc"]


def kernel(**inputs) -> np.ndarray:
    inputs = {k: np.asarray(v) for k, v in inputs.items()}
    in_maps = _prep_inputs(**inputs)
    nc = _get_nc()
    res = run_bass_kernel_spmd(nc, in_maps, core_ids=list(range(N_CORES)))
    y = sum(np.asarray(r["y"], np.float64) for r in res.results)
    y = y + np.asarray(inputs["b_out"], np.float64)
    return y.astype(np.float32)

